# revision 27
# baseline (speedup 1.0000x reference)
"""Trainium2 Bass kernel for CandidateFinder (retrieval_knn).

Math: for each (batch, query row), candidates = the K_MAX=64 smallest key
indices whose 32-dim sign pattern matches the query's in either dim-group
(dims 0:32 or 32:64), ascending, padded with -1.  This equals the
reference's per-group-topk + merge (per-group truncation can never drop an
index that would make the merged top-64).

Host side (sharding/layout only, no arithmetic): batch b and query-half
go to core 2b+half; only the high byte of each f32 is shipped (sign +
7 exponent bits; byte<128 <=> x>0 whenever |x| >= 2^-125, true for this
data) laid out dim-major: pairs of 128-row query slabs stacked into the
four 32-partition PE strips, keys replicated onto the upper 64
partitions, so four K=32 matmuls run concurrently in PE row-groups.

Device per core:
  - three parallel u8 DMA kicks (SP: queries, ACT: key chunks 0-1,
    POOL: chunks 2-3), 4x less HBM traffic than f32
  - mixed-scale quantize in the DMA shadow: DVE makes queries + key
    chunks 0-1 {+0.5,-0.5} bf16; ACT's Sign LUT (co-resident with Relu,
    no extra table load) makes chunks 2-3 {+1,-1}, so dots there span
    +-16 (match 16, runner-up 15) vs +-8 (match 8, runner-up 7.5)
  - all-pairs group dots in [128,1024] PSUM tiles; detection: 17 DVE
    row-max units (rstat cols 0:17) + 15 ACT relu-sum units (cols
    17:32), thresholded per scale via a 3-segment threshv vector
  - global any-match flag via ones-matmul partition reduce -> register;
    fast path ships all -1 early (gpsimd queue)
  - rare exact path (tc.If): recompute dots, per-chunk thresholds,
    extract 64 smallest via vector.max + match_replace, then re-DMA
"""

import numpy as np

import concourse.bacc as bacc
import concourse.mybir as mybir
from concourse.tile import TileContext
from concourse import bass_utils, bass_isa

B, L, D = 4, 2048, 64
HALF = 1024          # query rows per core
N_CORES = 8
K_MAX = 64
G = 32               # dims per group
QT = HALF // 128     # 8 query slabs per core
MATCH_DOT = 16.0     # queries +-1, keys chunks 0-1 +-0.5
THRESH = 15.5        # chunks 0-1: match dot 16, runner-up 15
THRESH2 = 31.0       # chunks 2-3 (+-1): match dot 32, runner-up 30
SENT = 4096.0        # sentinel > any index

f32 = mybir.dt.float32
bf16 = mybir.dt.bfloat16
i32 = mybir.dt.int32
u8 = mybir.dt.uint8
u32 = mybir.dt.uint32
Alu = mybir.AluOpType
Ax = mybir.AxisListType
AF = mybir.ActivationFunctionType

_CACHE = {}


def _build():
    nc = bacc.Bacc("TRN2", target_bir_lowering=False,
                   enable_partition_id=False)
    # qb4[h*64+d, pair*128+p] = sign byte of q[p*8 + 2*pair + h, d]
    # kb01 / kb23 = kb4[dup*64+d, j] = sign byte of k[j, d], split 1024|1024
    qb4 = nc.dram_tensor("qb4", [128, 512], u8, kind="ExternalInput")
    kb01 = nc.dram_tensor("kb01", [128, 1024], u8, kind="ExternalInput")
    kb23 = nc.dram_tensor("kb23", [128, 1024], u8, kind="ExternalInput")
    out = nc.dram_tensor("out", [HALF, K_MAX], i32, kind="ExternalOutput")
    out_pt = out[:].rearrange("(p t) c -> p (t c)", p=128)

    with TileContext(nc) as tc:
        with tc.tile_pool(name="sb", bufs=1) as sb, \
             tc.tile_pool(name="sb2", bufs=3) as sb2, \
             tc.tile_pool(name="ps", bufs=2, space="PSUM") as ps:

            # ---- load sign bytes (3 parallel kicks into one tile,
            # uniform 512-col alignment); quantize on DVE ----
            inb = sb.tile([128, 2560], u8)
            sqT4 = sb.tile([128, HALF // 2], bf16)
            skT4 = sb.tile([128, L], bf16)
            nc.default_dma_engine.dma_start(inb[:, 0:512], qb4[:, :])
            nc.scalar.dma_start(inb[:, 512:1536], kb01[:, :])
            nc.gpsimd.dma_start(inb[:, 1536:2560], kb23[:, :])
            # byte < 128 <=> sign bit clear <=> x > 0.  DVE quantizes
            # queries + key chunks 0-1 to +-0.5; ACT (Sign LUT, co-resident
            # with Relu) takes chunks 2-3 to +-1 -> dots there span +-16.
            kbias = sb.tile([128, 1], f32)
            nc.vector.memset(kbias, 127.5)
            # queries +-1 on ACT Sign (rides the earliest kick, so ACT
            # starts while DVE only handles key chunks 0-1 at +-0.5)
            nc.scalar.activation(sqT4, inb[:, 0:512], AF.Sign,
                                 bias=kbias[:, 0:1], scale=-1.0)
            for c in range(2):
                nc.vector.tensor_scalar(
                    skT4[:, c * 512:(c + 1) * 512],
                    inb[:, 512 + c * 512:1024 + c * 512], 128.0, 0.5,
                    op0=Alu.is_lt, op1=Alu.subtract)
            for c in range(2, 4):
                nc.scalar.activation(
                    skT4[:, c * 512:(c + 1) * 512],
                    inb[:, 512 + c * 512:1024 + c * 512], AF.Sign,
                    bias=kbias[:, 0:1], scale=-1.0)

            # ---- early fast-path output: all -1 (gpsimd memset + kick) ----
            out_sb = sb.tile([128, QT * K_MAX], i32)
            nc.gpsimd.memset(out_sb, -1)
            nc.gpsimd.dma_start(out_pt, out_sb)

            # ---- all-pairs dots, 4 concurrent K=32 matmuls ----
            # rstat layout: DVE max cols 0:17 (0:8 scale-8, 8:17
            # scale-16), ACT relu-sum cols 17:32 (always 0 or >= 8)
            rstat = sb.tile([128, 32], f32)
            rbias = sb.tile([128, 1], f32)
            nc.vector.memset(rbias, -80.0 * THRESH)
            rbias2 = sb.tile([128, 1], f32)
            nc.vector.memset(rbias2, -80.0 * THRESH2)
            threshv = sb.tile([128, 32], f32)
            nc.vector.memset(threshv[:, 0:8], THRESH)
            nc.vector.memset(threshv[:, 8:17], THRESH2)
            nc.vector.memset(threshv[:, 17:32], THRESH)
            for ns in range(4):
                kc = slice(ns * 512, (ns + 1) * 512)
                for pair in range(QT // 2):
                    it = ns * 4 + pair
                    qc = slice(pair * 128, (pair + 1) * 128)
                    pG0 = ps.tile([128, 1024], f32, tag="g0")
                    pG1 = ps.tile([128, 1024], f32, tag="g1")
                    nc.tensor.matmul(pG0[:, 0:512], lhsT=sqT4[0:32, qc],
                                     rhs=skT4[0:32, kc], start=True,
                                     stop=True, tile_position=(0, 0))
                    nc.tensor.matmul(pG1[:, 0:512], lhsT=sqT4[32:64, qc],
                                     rhs=skT4[32:64, kc], start=True,
                                     stop=True, tile_position=(32, 0))
                    nc.tensor.matmul(pG0[:, 512:1024], lhsT=sqT4[64:96, qc],
                                     rhs=skT4[64:96, kc], start=True,
                                     stop=True, tile_position=(64, 0))
                    nc.tensor.matmul(pG1[:, 512:1024], lhsT=sqT4[96:128, qc],
                                     rhs=skT4[96:128, kc], start=True,
                                     stop=True, tile_position=(96, 0))
                    # detection: 17 units on DVE, 15 on ACT (ACT trails the
                    # pipeline, so the final g1 tile goes to DVE)
                    for g, pg in ((0, pG0), (1, pG1)):
                        if (g == 1) and it != 15:
                            col = 17 + it
                            scr = sb2.tile([128, 1024], bf16, tag="scr")
                            nc.scalar.activation(
                                scr, pg, AF.Relu,
                                bias=(rbias if ns < 2 else rbias2)[:, 0:1],
                                scale=80.0,
                                accum_out=rstat[:, col:col + 1])
                        else:
                            col = it if g == 0 else 16
                            nc.vector.tensor_reduce(
                                rstat[:, col:col + 1], pg,
                                axis=Ax.X, op=Alu.max)

            # ---- scalar any-match flag ----
            ones = sb.tile([128, 1], f32)
            nc.vector.memset(ones, 1.0)
            srfv = sb.tile([128, 32], f32)
            nc.vector.tensor_tensor(out=srfv, in0=rstat, in1=threshv,
                                    op=Alu.is_ge)
            srf = sb.tile([128, 1], f32)
            nc.vector.tensor_reduce(srf, srfv, axis=Ax.X, op=Alu.max)
            fps = ps.tile([1, 1], f32, tag="g0")
            nc.tensor.matmul(fps, lhsT=ones, rhs=srf, start=True, stop=True)
            flag = sb.tile([1, 1], i32)
            nc.vector.tensor_scalar(flag, fps[0:1, 0:1], 0.5, None,
                                    op0=Alu.is_ge)
            rv = nc.values_load(
                flag[0:1, 0:1], min_val=0, max_val=1,
                skip_runtime_bounds_check=True,
                engines=(mybir.EngineType.PE, mybir.EngineType.DVE,
                         mybir.EngineType.Pool, mybir.EngineType.SP))

            # ---- rare exact path (recompute + extract, then re-DMA) ----
            with tc.If(rv > 0):
                c2i = sb.tile([128, L], i32)   # SENT - j (key j = column)
                nc.gpsimd.iota(c2i, pattern=[[-1, L]], base=int(SENT),
                               channel_multiplier=0)
                c2f = sb.tile([128, L], f32)
                nc.gpsimd.tensor_copy(c2f, c2i)
                negone = sb.tile([128, K_MAX], f32)
                nc.vector.memset(negone, -1.0)
                for t in range(QT):
                    base = (t % 2) * 64
                    qc = slice((t // 2) * 128, (t // 2) * 128 + 128)
                    lhs0 = sqT4[base:base + 32, qc]
                    lhs1 = sqT4[base + 32:base + 64, qc]
                    val = sb.tile([128, L], f32, tag="val")
                    for h in range(2):
                        p0 = ps.tile([128, 1024], f32, tag="g0")
                        p1 = ps.tile([128, 1024], f32, tag="g1")
                        for s in range(2):
                            kc = slice(h * 1024 + s * 512,
                                       h * 1024 + (s + 1) * 512)
                            sl = slice(s * 512, (s + 1) * 512)
                            nc.tensor.matmul(p0[:, sl], lhsT=lhs0,
                                             rhs=skT4[base:base + 32, kc],
                                             start=True, stop=True,
                                             tile_position=(base, 0))
                            nc.tensor.matmul(p1[:, sl], lhsT=lhs1,
                                             rhs=skT4[base + 32:base + 64,
                                                      kc],
                                             start=True, stop=True,
                                             tile_position=(base + 32, 0))
                        thr = THRESH if h == 0 else THRESH2
                        hsl = slice(h * 1024, (h + 1) * 1024)
                        m0 = sb2.tile([128, 1024], f32, tag="m0")
                        nc.vector.tensor_scalar(m0, p0, thr,
                                                None, op0=Alu.is_ge)
                        m1 = sb2.tile([128, 1024], f32, tag="m1")
                        nc.vector.scalar_tensor_tensor(
                            m1, in0=p1, scalar=thr, in1=m0,
                            op0=Alu.is_ge, op1=Alu.max)
                        # val = m1 ? -(j) : -SENT  ==  m1*(SENT-j) - SENT
                        nc.vector.tensor_tensor(
                            out=val[:, hsl], in0=m1, in1=c2f[:, hsl],
                            op=Alu.mult)
                        nc.vector.tensor_scalar_add(val[:, hsl], val[:, hsl],
                                                    -SENT)
                    # 64 smallest j == 64 largest of val, descending
                    no = sb.tile([128, K_MAX], f32, tag="no")
                    for it8 in range(8):
                        osl = slice(it8 * 8, (it8 + 1) * 8)
                        nc.vector.max(out=no[:, osl], in_=val)
                        nc.vector.match_replace(
                            out=val, in_to_replace=no[:, osl],
                            in_values=val, imm_value=-SENT)
                    jv = sb.tile([128, K_MAX], f32, tag="jv")
                    nc.vector.tensor_scalar_mul(jv, no, -1.0)  # j or SENT
                    msk = sb.tile([128, K_MAX], u32, tag="msk")
                    nc.vector.tensor_scalar(msk, jv, 2048.5, None,
                                            op0=Alu.is_ge)
                    nc.vector.copy_predicated(jv, msk, negone)
                    nc.vector.tensor_copy(
                        out_sb[:, t * K_MAX:(t + 1) * K_MAX], jv)
                # overwrite the early -1s with the exact candidates
                nc.default_dma_engine.dma_start(out_pt, out_sb)

    nc.compile()
    return nc


def get_nc():
    if "nc" not in _CACHE:
        _CACHE["nc"] = _build()
    return _CACHE["nc"]


def _sign_bytes(x):
    """High byte of each f32 (pure byte-level layout transform)."""
    x = np.ascontiguousarray(x, dtype=np.float32)
    return x.view(np.uint8).reshape(x.shape + (4,))[..., 3]


def make_in_maps(query_up, key_up):
    """Pure layout transforms (byte-slice/transpose/replicate) per core."""
    qb = _sign_bytes(np.asarray(query_up, dtype=np.float32))  # [B, L, D] u8
    kbb = _sign_bytes(np.asarray(key_up, dtype=np.float32))   # [B, L, D] u8
    in_maps = []
    for c in range(N_CORES):
        b, half = c // 2, c % 2
        q = qb[b, half * HALF:(half + 1) * HALF]             # [1024, 64] u8
        # [p, pair, h, d] -> [h, d, pair, p] -> [128, 512]
        qb4 = np.ascontiguousarray(
            q.reshape(128, 4, 2, D).transpose(2, 3, 1, 0).reshape(
                128, HALF // 2))
        kT = kbb[b].T                                        # [64, 2048] u8
        kb4 = np.concatenate([kT, kT], axis=0)               # [128, 2048]
        in_maps.append({
            "qb4": qb4,
            "kb01": np.ascontiguousarray(kb4[:, 0:1024]),
            "kb23": np.ascontiguousarray(kb4[:, 1024:2048]),
        })
    return in_maps


def kernel(query_up, key_up, head_idx=None, **_ignored):
    nc = get_nc()
    in_maps = make_in_maps(query_up, key_up)
    res = bass_utils.run_bass_kernel_spmd(
        nc, in_maps, core_ids=list(range(N_CORES)))
    full = np.empty((B, L, K_MAX), dtype=np.int32)
    for c in range(N_CORES):
        b, half = c // 2, c % 2
        # out row p*8 + t <-> query row p*8 + 2*(t//2) + t%2 == p*8 + t
        full[b, half * HALF:(half + 1) * HALF] = res.results[c]["out"]
    return full


# revision 28
# speedup vs baseline: 1.0159x; 1.0159x over previous
"""Trainium2 Bass kernel for CandidateFinder (retrieval_knn).

Math: for each (batch, query row), candidates = the K_MAX=64 smallest key
indices whose 32-dim sign pattern matches the query's in either dim-group
(dims 0:32 or 32:64), ascending, padded with -1.  This equals the
reference's per-group-topk + merge (per-group truncation can never drop an
index that would make the merged top-64).

Host side (sharding/layout only, no arithmetic): batch b and query-half
go to core 2b+half; only the high byte of each f32 is shipped (sign +
7 exponent bits; byte<128 <=> x>0 whenever |x| >= 2^-125, true for this
data) laid out dim-major: pairs of 128-row query slabs stacked into the
four 32-partition PE strips, keys replicated onto the upper 64
partitions, so four K=32 matmuls run concurrently in PE row-groups.

Device per core:
  - three parallel u8 DMA kicks (SP: queries, ACT: key chunks 0-1,
    POOL: chunks 2-3), 4x less HBM traffic than f32
  - mixed-scale quantize in the DMA shadow: DVE makes queries + key
    chunks 0-1 {+0.5,-0.5} bf16; ACT's Sign LUT (co-resident with Relu,
    no extra table load) makes chunks 2-3 {+1,-1}, so dots there span
    +-16 (match 16, runner-up 15) vs +-8 (match 8, runner-up 7.5)
  - all-pairs group dots in [128,1024] PSUM tiles; detection: 17 DVE
    row-max units (rstat cols 0:17) + 15 ACT relu-sum units (cols
    17:32), thresholded per scale via a 3-segment threshv vector
  - global any-match flag via ones-matmul partition reduce -> register;
    fast path ships all -1 early (gpsimd queue)
  - rare exact path (tc.If): recompute dots, per-chunk thresholds,
    extract 64 smallest via vector.max + match_replace, then re-DMA
"""

import numpy as np

import concourse.bacc as bacc
import concourse.mybir as mybir
from concourse.tile import TileContext
from concourse import bass_utils, bass_isa

B, L, D = 4, 2048, 64
HALF = 1024          # query rows per core
N_CORES = 8
K_MAX = 64
G = 32               # dims per group
QT = HALF // 128     # 8 query slabs per core
MATCH_DOT = 8.0      # 32 * 0.5 * 0.5
THRESH = 7.9         # keys +-0.5 (chunks 0-1): match dot 8, runner-up 7.5
THRESH2 = 15.5       # keys +-1  (chunks 2-3): match dot 16, runner-up 15
SENT = 4096.0        # sentinel > any index

f32 = mybir.dt.float32
bf16 = mybir.dt.bfloat16
i32 = mybir.dt.int32
u8 = mybir.dt.uint8
u32 = mybir.dt.uint32
Alu = mybir.AluOpType
Ax = mybir.AxisListType
AF = mybir.ActivationFunctionType

_CACHE = {}


def _build():
    nc = bacc.Bacc("TRN2", target_bir_lowering=False,
                   enable_partition_id=False)
    # qb4[h*64+d, pair*128+p] = sign byte of q[p*8 + 2*pair + h, d]
    # kb01 / kb23 = kb4[dup*64+d, j] = sign byte of k[j, d], split 1024|1024
    qb4 = nc.dram_tensor("qb4", [128, 512], u8, kind="ExternalInput")
    kb01 = nc.dram_tensor("kb01", [128, 1024], u8, kind="ExternalInput")
    kb23 = nc.dram_tensor("kb23", [128, 1024], u8, kind="ExternalInput")
    out = nc.dram_tensor("out", [HALF, K_MAX], i32, kind="ExternalOutput")
    out_pt = out[:].rearrange("(p t) c -> p (t c)", p=128)

    with TileContext(nc) as tc:
        with tc.tile_pool(name="sb", bufs=1) as sb, \
             tc.tile_pool(name="sb2", bufs=3) as sb2, \
             tc.tile_pool(name="ps", bufs=2, space="PSUM") as ps:

            # ---- load sign bytes (3 parallel kicks into one tile,
            # uniform 512-col alignment); quantize on DVE ----
            inb = sb.tile([128, 2560], u8)
            sqT4 = sb.tile([128, HALF // 2], bf16)
            skT4 = sb.tile([128, L], bf16)
            nc.default_dma_engine.dma_start(inb[:, 0:512], qb4[:, :])
            nc.scalar.dma_start(inb[:, 512:1536], kb01[:, :])
            nc.gpsimd.dma_start(inb[:, 1536:2560], kb23[:, :])
            # byte < 128 <=> sign bit clear <=> x > 0.  DVE quantizes
            # queries + key chunks 0-1 to +-0.5; ACT (Sign LUT, co-resident
            # with Relu) takes chunks 2-3 to +-1 -> dots there span +-16.
            nc.vector.tensor_scalar(sqT4, inb[:, 0:512], 128.0, 0.5,
                                    op0=Alu.is_lt, op1=Alu.subtract)
            for c in range(2):
                nc.vector.tensor_scalar(
                    skT4[:, c * 512:(c + 1) * 512],
                    inb[:, 512 + c * 512:1024 + c * 512], 128.0, 0.5,
                    op0=Alu.is_lt, op1=Alu.subtract)
            kbias = sb.tile([128, 1], f32)
            nc.vector.memset(kbias, 127.5)
            for c in range(2, 4):
                nc.scalar.activation(
                    skT4[:, c * 512:(c + 1) * 512],
                    inb[:, 512 + c * 512:1024 + c * 512], AF.Sign,
                    bias=kbias[:, 0:1], scale=-1.0)

            # ---- early fast-path output: all -1 (gpsimd memset + kick) ----
            out_sb = sb.tile([128, QT * K_MAX], i32)
            nc.gpsimd.memset(out_sb, -1)
            nc.gpsimd.dma_start(out_pt, out_sb)

            # ---- all-pairs dots, 4 concurrent K=32 matmuls ----
            # rstat layout: DVE max cols 0:17 (0:8 scale-8, 8:17
            # scale-16), ACT relu-sum cols 17:32 (always 0 or >= 8)
            rstat = sb.tile([128, 32], f32)
            rbias = sb.tile([128, 1], f32)
            nc.vector.memset(rbias, -80.0 * THRESH)
            rbias2 = sb.tile([128, 1], f32)
            nc.vector.memset(rbias2, -80.0 * THRESH2)
            threshv = sb.tile([128, 32], f32)
            nc.vector.memset(threshv[:, 0:8], THRESH)
            nc.vector.memset(threshv[:, 8:17], THRESH2)
            nc.vector.memset(threshv[:, 17:32], THRESH)
            for ns in range(4):
                kc = slice(ns * 512, (ns + 1) * 512)
                for pair in range(QT // 2):
                    it = ns * 4 + pair
                    qc = slice(pair * 128, (pair + 1) * 128)
                    pG0 = ps.tile([128, 1024], f32, tag="g0")
                    pG1 = ps.tile([128, 1024], f32, tag="g1")
                    nc.tensor.matmul(pG0[:, 0:512], lhsT=sqT4[0:32, qc],
                                     rhs=skT4[0:32, kc], start=True,
                                     stop=True, tile_position=(0, 0))
                    nc.tensor.matmul(pG1[:, 0:512], lhsT=sqT4[32:64, qc],
                                     rhs=skT4[32:64, kc], start=True,
                                     stop=True, tile_position=(32, 0))
                    nc.tensor.matmul(pG0[:, 512:1024], lhsT=sqT4[64:96, qc],
                                     rhs=skT4[64:96, kc], start=True,
                                     stop=True, tile_position=(64, 0))
                    nc.tensor.matmul(pG1[:, 512:1024], lhsT=sqT4[96:128, qc],
                                     rhs=skT4[96:128, kc], start=True,
                                     stop=True, tile_position=(96, 0))
                    # detection: 17 units on DVE, 15 on ACT (ACT trails the
                    # pipeline, so the final g1 tile goes to DVE)
                    for g, pg in ((0, pG0), (1, pG1)):
                        if (g == 1) and it != 15:
                            col = 17 + it
                            scr = sb2.tile([128, 1024], bf16, tag="scr")
                            nc.scalar.activation(
                                scr, pg, AF.Relu,
                                bias=(rbias if ns < 2 else rbias2)[:, 0:1],
                                scale=80.0,
                                accum_out=rstat[:, col:col + 1])
                        else:
                            col = it if g == 0 else 16
                            nc.vector.tensor_reduce(
                                rstat[:, col:col + 1], pg,
                                axis=Ax.X, op=Alu.max)

            # ---- scalar any-match flag ----
            ones = sb.tile([128, 1], f32)
            nc.vector.memset(ones, 1.0)
            srfv = sb.tile([128, 32], f32)
            nc.vector.tensor_tensor(out=srfv, in0=rstat, in1=threshv,
                                    op=Alu.is_ge)
            srf = sb.tile([128, 1], f32)
            nc.vector.tensor_reduce(srf, srfv, axis=Ax.X, op=Alu.max)
            fps = ps.tile([1, 1], f32, tag="g0")
            nc.tensor.matmul(fps, lhsT=ones, rhs=srf, start=True, stop=True)
            flag = sb.tile([1, 1], i32)
            nc.vector.tensor_scalar(flag, fps[0:1, 0:1], 0.5, None,
                                    op0=Alu.is_ge)
            rv = nc.values_load(
                flag[0:1, 0:1], min_val=0, max_val=1,
                skip_runtime_bounds_check=True,
                engines=(mybir.EngineType.PE, mybir.EngineType.DVE,
                         mybir.EngineType.Pool, mybir.EngineType.SP))

            # ---- rare exact path (recompute + extract, then re-DMA) ----
            with tc.If(rv > 0):
                c2i = sb.tile([128, L], i32)   # SENT - j (key j = column)
                nc.gpsimd.iota(c2i, pattern=[[-1, L]], base=int(SENT),
                               channel_multiplier=0)
                c2f = sb.tile([128, L], f32)
                nc.gpsimd.tensor_copy(c2f, c2i)
                negone = sb.tile([128, K_MAX], f32)
                nc.vector.memset(negone, -1.0)
                for t in range(QT):
                    base = (t % 2) * 64
                    qc = slice((t // 2) * 128, (t // 2) * 128 + 128)
                    lhs0 = sqT4[base:base + 32, qc]
                    lhs1 = sqT4[base + 32:base + 64, qc]
                    val = sb.tile([128, L], f32, tag="val")
                    for h in range(2):
                        p0 = ps.tile([128, 1024], f32, tag="g0")
                        p1 = ps.tile([128, 1024], f32, tag="g1")
                        for s in range(2):
                            kc = slice(h * 1024 + s * 512,
                                       h * 1024 + (s + 1) * 512)
                            sl = slice(s * 512, (s + 1) * 512)
                            nc.tensor.matmul(p0[:, sl], lhsT=lhs0,
                                             rhs=skT4[base:base + 32, kc],
                                             start=True, stop=True,
                                             tile_position=(base, 0))
                            nc.tensor.matmul(p1[:, sl], lhsT=lhs1,
                                             rhs=skT4[base + 32:base + 64,
                                                      kc],
                                             start=True, stop=True,
                                             tile_position=(base + 32, 0))
                        thr = THRESH if h == 0 else THRESH2
                        hsl = slice(h * 1024, (h + 1) * 1024)
                        m0 = sb2.tile([128, 1024], f32, tag="m0")
                        nc.vector.tensor_scalar(m0, p0, thr,
                                                None, op0=Alu.is_ge)
                        m1 = sb2.tile([128, 1024], f32, tag="m1")
                        nc.vector.scalar_tensor_tensor(
                            m1, in0=p1, scalar=thr, in1=m0,
                            op0=Alu.is_ge, op1=Alu.max)
                        # val = m1 ? -(j) : -SENT  ==  m1*(SENT-j) - SENT
                        nc.vector.tensor_tensor(
                            out=val[:, hsl], in0=m1, in1=c2f[:, hsl],
                            op=Alu.mult)
                        nc.vector.tensor_scalar_add(val[:, hsl], val[:, hsl],
                                                    -SENT)
                    # 64 smallest j == 64 largest of val, descending
                    no = sb.tile([128, K_MAX], f32, tag="no")
                    for it8 in range(8):
                        osl = slice(it8 * 8, (it8 + 1) * 8)
                        nc.vector.max(out=no[:, osl], in_=val)
                        nc.vector.match_replace(
                            out=val, in_to_replace=no[:, osl],
                            in_values=val, imm_value=-SENT)
                    jv = sb.tile([128, K_MAX], f32, tag="jv")
                    nc.vector.tensor_scalar_mul(jv, no, -1.0)  # j or SENT
                    msk = sb.tile([128, K_MAX], u32, tag="msk")
                    nc.vector.tensor_scalar(msk, jv, 2048.5, None,
                                            op0=Alu.is_ge)
                    nc.vector.copy_predicated(jv, msk, negone)
                    nc.vector.tensor_copy(
                        out_sb[:, t * K_MAX:(t + 1) * K_MAX], jv)
                # overwrite the early -1s with the exact candidates
                nc.default_dma_engine.dma_start(out_pt, out_sb)

    nc.compile()
    return nc


def get_nc():
    if "nc" not in _CACHE:
        _CACHE["nc"] = _build()
    return _CACHE["nc"]


def _sign_bytes(x):
    """High byte of each f32 (pure byte-level layout transform)."""
    x = np.ascontiguousarray(x, dtype=np.float32)
    return x.view(np.uint8).reshape(x.shape + (4,))[..., 3]


def make_in_maps(query_up, key_up):
    """Pure layout transforms (byte-slice/transpose/replicate) per core."""
    qb = _sign_bytes(np.asarray(query_up, dtype=np.float32))  # [B, L, D] u8
    kbb = _sign_bytes(np.asarray(key_up, dtype=np.float32))   # [B, L, D] u8
    in_maps = []
    for c in range(N_CORES):
        b, half = c // 2, c % 2
        q = qb[b, half * HALF:(half + 1) * HALF]             # [1024, 64] u8
        # [p, pair, h, d] -> [h, d, pair, p] -> [128, 512]
        qb4 = np.ascontiguousarray(
            q.reshape(128, 4, 2, D).transpose(2, 3, 1, 0).reshape(
                128, HALF // 2))
        kT = kbb[b].T                                        # [64, 2048] u8
        kb4 = np.concatenate([kT, kT], axis=0)               # [128, 2048]
        in_maps.append({
            "qb4": qb4,
            "kb01": np.ascontiguousarray(kb4[:, 0:1024]),
            "kb23": np.ascontiguousarray(kb4[:, 1024:2048]),
        })
    return in_maps


def kernel(query_up, key_up, head_idx=None, **_ignored):
    nc = get_nc()
    in_maps = make_in_maps(query_up, key_up)
    res = bass_utils.run_bass_kernel_spmd(
        nc, in_maps, core_ids=list(range(N_CORES)))
    full = np.empty((B, L, K_MAX), dtype=np.int32)
    for c in range(N_CORES):
        b, half = c // 2, c % 2
        # out row p*8 + t <-> query row p*8 + 2*(t//2) + t%2 == p*8 + t
        full[b, half * HALF:(half + 1) * HALF] = res.results[c]["out"]
    return full
